# revision 12
# baseline (speedup 1.0000x reference)
"""DeepSeek MoE gate (noaux_tc routing) on 8 TRN2 NeuronCores.

Full inputs:
    hidden_states            [8192, 7168] f32
    weight                   [256, 7168]  f32
    e_score_correction_bias  [256]        f32
Full outputs (tuple, like the reference):
    routing_weights          [8192, 8] f32
    selected_experts         [8192, 8] int32

Sharding: token dim split 8 ways (1024 tokens/core); weight + bias replicated.

Matmul scheme: f32r3 (xh@wh + xl@wh + xh@wl with f32r operands, xh=f32r(x),
xl = x-xh). Error ~2^-26: fp32-class top-k accuracy at 3 cycles/row.

v2 layout vs the original:
  - wT stored stacked [h, k, 512] = [whT | wlT] so the xh passes issue as ONE
    512-row moving matmul per k-chunk (amortizes LDWEIGHTS + sequencer issue
    overhead: 2 matmuls/chunk instead of 3).
  - xh@[wh|wl] accumulates in psA [128,512]; xl@wh in psB [128,256];
    logits folded with two DVE adds in the epilogue.
  - x streamed in quarter-tiles with a full token-tile of prefetch
    (pool bufs=8) so the PE never waits on HBM at tile boundaries.
  - w prep (14 chunks) interleaved into tile 0's transpose pipeline so the
    PE starts on x as soon as the first x quarter lands.
  - outputs DMA'd per-tile instead of once at the end (shorter tail).
"""

import os
import sys

import numpy as np

T_FULL = 8192
H = 7168
E = 256
N_CORES = 8
T_LOC = T_FULL // N_CORES          # 1024 tokens per core
P = 128                            # partition tile
N_TTILES = T_LOC // P              # 8 token tiles per core
N_K = H // P                       # 56 contraction chunks
QUART = H // 4                     # 1792 cols per x quarter-tile
KQ = N_K // 4                      # 14 k-chunks per x quarter
N_GROUP = 8
EG = E // N_GROUP                  # 32 experts per group
TOPK_GROUP = 4
TOP_K = 8
ROUTED_SCALING = 2.5

N_WCHUNK = 14                      # w loaded in 14 chunks of 512 cols
WCOL = H // N_WCHUNK               # 512
KC = N_K // N_WCHUNK               # 4 k-chunks per w chunk
N_WBUF = 4                         # w chunks in flight

SCHEME = "f32r3s"                  # stacked-moving f32r3


def _ensure_path():
    for p in ("/opt/trn_rl_repo", "/root/.axon_site/_ro/trn_rl_repo"):
        if os.path.isdir(p) and p not in sys.path:
            sys.path.append(p)


def _build_program():
    _ensure_path()
    import concourse.bass as bass  # noqa: F401
    import concourse.mybir as mybir
    from concourse import bacc
    from concourse.tile import TileContext

    f32 = mybir.dt.float32
    f32r = mybir.dt.float32r
    u32 = mybir.dt.uint32
    i32 = mybir.dt.int32
    Alu = mybir.AluOpType
    Act = mybir.ActivationFunctionType

    nc = bacc.Bacc("TRN2", debug=False, enable_asserts=False)

    hs = nc.dram_tensor("hidden_states", [T_LOC, H], f32, kind="ExternalInput")
    wt = nc.dram_tensor("weight", [E, H], f32, kind="ExternalInput")
    bias = nc.dram_tensor(
        "e_score_correction_bias", [E], f32, kind="ExternalInput"
    )
    out_w = nc.dram_tensor("routing_weights", [T_LOC, TOP_K], f32, kind="ExternalOutput")
    out_i = nc.dram_tensor("selected_experts", [T_LOC, TOP_K], i32, kind="ExternalOutput")

    with TileContext(nc) as tc:
        with (
            tc.tile_pool(name="const", bufs=1) as const_pool,
            tc.tile_pool(name="wT", bufs=1) as wT_pool,
            tc.tile_pool(name="wnat", bufs=2) as wnat_pool,
            tc.tile_pool(name="x", bufs=8) as x_pool,
            tc.tile_pool(name="xt", bufs=6) as xt_pool,
            tc.tile_pool(name="ps_t", bufs=3, space="PSUM") as ps_t_pool,
            tc.tile_pool(name="ps_a", bufs=2, space="PSUM") as ps_a_pool,
            tc.tile_pool(name="ps_b", bufs=2, space="PSUM") as ps_b_pool,
            tc.tile_pool(name="epi", bufs=2) as epi_pool,
        ):
            # ---- constants (tiny DMAs first) -------------------------------
            eye_dram = nc.inline_tensor(np.eye(P, dtype=np.float32), name="eye128")
            identity = const_pool.tile([P, P], f32)
            nc.sync.dma_start(out=identity, in_=eye_dram.ap())

            ones_dram = nc.inline_tensor(
                np.ones((1, P), dtype=np.float32), name="ones128"
            )
            ones_row = const_pool.tile([1, P], f32)
            nc.sync.dma_start(out=ones_row, in_=ones_dram.ap())

            bias_row = const_pool.tile([1, E], f32)
            nc.sync.dma_start(
                out=bias_row, in_=bias.ap().rearrange("(o e) -> o e", o=1)
            )

            # ---- x quarter-tile streaming ----------------------------------
            x_quart = {}

            def load_quart(ti, q):
                xq = x_pool.tile([P, QUART], f32, tag="x", name=f"x_{ti}_{q}")
                x_quart[(ti, q)] = xq
                nc.sync.dma_start(
                    out=xq,
                    in_=hs.ap()[
                        ti * P : (ti + 1) * P,
                        q * QUART : (q + 1) * QUART,
                    ],
                )

            def load_tile(ti):
                for q in range(4):
                    load_quart(ti, q)

            # ---- wT stacked [h, k, 0:256]=whT, [256:512]=wlT ---------------
            # DMA loads (wload) decoupled from PE transposes (wxform) so the
            # weight stream overlaps tile-0's x pipeline.
            wT = wT_pool.tile([P, N_K, 2 * E], f32r)
            w_nat_bufs = {}

            def wload(q):
                w_nat = wnat_pool.tile([P, 2, WCOL], f32, tag="wnat")
                w_nat_bufs[q] = w_nat
                for eh in range(2):
                    nc.sync.dma_start(
                        out=w_nat[:, eh, :],
                        in_=wt.ap()[
                            eh * P : (eh + 1) * P,
                            q * WCOL : (q + 1) * WCOL,
                        ],
                    )

            def wxform(q):
                w_nat = w_nat_bufs.pop(q)
                for kk in range(KC):
                    k = q * KC + kk
                    pst = ps_t_pool.tile([P, 2 * P], f32, tag="ps_t")
                    for eh in range(2):
                        nc.tensor.transpose(
                            pst[:, eh * P : (eh + 1) * P],
                            w_nat[:, eh, kk * P : (kk + 1) * P],
                            identity,
                        )
                    nc.scalar.copy(wT[:, k, 0:E], pst)
                    # wl = w - f32r(w); exactly representable in f32r
                    nc.vector.scalar_tensor_tensor(
                        out=wT[:, k, E : 2 * E],
                        in0=wT[:, k, 0:E],
                        scalar=-1.0,
                        in1=pst,
                        op0=Alu.mult,
                        op1=Alu.add,
                    )

            # interleave tile-0 x quarters with the first w chunks on the
            # (FIFO) DMA queue so neither stream starves the PE early
            load_quart(0, 0)
            wload(0)
            wload(1)
            load_quart(0, 1)
            wload(2)
            wload(3)
            load_quart(0, 2)
            load_quart(0, 3)

            # broadcast bias across partitions: rank-1 matmul ones^T @ bias_row
            bias_bc = const_pool.tile([P, E], f32)
            ps_bias = ps_b_pool.tile([P, E], f32, tag="ps_b")
            nc.tensor.matmul(ps_bias, lhsT=ones_row, rhs=bias_row, start=True, stop=True)
            nc.vector.tensor_copy(bias_bc, ps_bias)

            # ---- main loop over token tiles --------------------------------
            for ti in range(N_TTILES):
                if 1 <= ti < N_TTILES - 1:
                    load_tile(ti + 1)

                psA = ps_a_pool.tile([P, 2 * E], f32, tag="ps_a")

                # software-pipelined: transpose pair pk+1 while pair pk matmuls
                n_pairs = N_K // 2
                pend = []  # (k0, xh_tile, xl_tile)
                for pk in range(n_pairs + 1):
                    if pk < n_pairs:
                        k0 = 2 * pk
                        pst = ps_t_pool.tile([P, 2 * P], f32, tag="ps_t")
                        for j in range(2):
                            k = k0 + j
                            src = x_quart[(ti, k // KQ)]
                            kk = k % KQ
                            nc.tensor.transpose(
                                pst[:, j * P : (j + 1) * P],
                                src[:, kk * P : (kk + 1) * P],
                                identity,
                            )
                        xh = xt_pool.tile([P, 2 * P], f32r, tag="xh")
                        nc.scalar.copy(xh, pst)  # ACT: rounds f32 -> f32r
                        xl = xt_pool.tile([P, 2 * P], f32r, tag="xl")
                        nc.vector.scalar_tensor_tensor(
                            out=xl,
                            in0=xh,
                            scalar=-1.0,
                            in1=pst,
                            op0=Alu.mult,
                            op1=Alu.add,
                        )
                        pend.append((k0, xh, xl))
                    if pk >= 1:
                        k0, xh, xl = pend[pk - 1]
                        for j in range(2):
                            k = k0 + j
                            first = k == 0
                            last = k == N_K - 1
                            xh_j = xh[:, j * P : (j + 1) * P]
                            xl_j = xl[:, j * P : (j + 1) * P]
                            # xh @ [wh | wl]: one 512-row moving matmul.
                            # xl @ wh accumulates into the same psum region
                            # [0:256] as xh @ wh (free PSUM accumulation);
                            # the final logits fold is [0:256] + [256:512].
                            nc.tensor.matmul(
                                psA, lhsT=xh_j, rhs=wT[:, k, :],
                                start=first, stop=False,
                            )
                            nc.tensor.matmul(
                                psA[:, 0:E], lhsT=xl_j, rhs=wT[:, k, 0:E],
                                start=False, stop=last,
                            )
                    if ti == 0:
                        if pk < N_WCHUNK:
                            wxform(pk)
                            if pk + N_WBUF < N_WCHUNK:
                                wload(pk + N_WBUF)
                        elif N_WCHUNK <= pk < N_WCHUNK + 4:
                            # tile-1 prefetch, queued after all w loads
                            load_quart(1, pk - N_WCHUNK)

                # ---- epilogue ---------------------------------------------
                # logits = psA[:, 0:256] + psA[:, 256:512]
                # (DVE may read only one PSUM operand -> ACT evacuates one half)
                lhalf = epi_pool.tile([P, E], f32, tag="lhalf")
                nc.scalar.copy(lhalf, psA[:, E : 2 * E])
                logits = epi_pool.tile([P, E], f32, tag="logits")
                nc.vector.tensor_add(logits, psA[:, 0:E], lhalf)

                scores = epi_pool.tile([P, E], f32, tag="scores")
                nc.scalar.activation(scores, logits, Act.Sigmoid)

                s_choice = epi_pool.tile([P, E], f32, tag="s_choice")
                nc.vector.tensor_add(s_choice, scores, bias_bc)

                # per-group top-8 (entries 0,1 used) -> group scores
                gmax = epi_pool.tile([P, N_GROUP, 8], f32, tag="gmax")
                for g in range(N_GROUP):
                    nc.vector.max(
                        out=gmax[:, g, :], in_=s_choice[:, g * EG : (g + 1) * EG]
                    )
                gscore = epi_pool.tile([P, N_GROUP], f32, tag="gscore")
                nc.vector.tensor_add(gscore, gmax[:, :, 0], gmax[:, :, 1])

                # top-4 groups: threshold at 4th largest group score
                g8 = epi_pool.tile([P, 8], f32, tag="g8")
                nc.vector.max(out=g8, in_=gscore)
                gmask = epi_pool.tile([P, N_GROUP], f32, tag="gmask")
                nc.vector.tensor_tensor(
                    out=gmask,
                    in0=gscore,
                    in1=g8[:, TOPK_GROUP - 1 : TOPK_GROUP].to_broadcast(
                        [P, N_GROUP]
                    ),
                    op=Alu.is_ge,
                )

                # expand to expert mask and apply
                emask = epi_pool.tile([P, E], f32, tag="emask")
                nc.vector.tensor_copy(
                    emask.rearrange("p (g x) -> p g x", g=N_GROUP),
                    gmask.rearrange("p (g x) -> p g x", x=1).to_broadcast(
                        [P, N_GROUP, EG]
                    ),
                )
                masked = epi_pool.tile([P, E], f32, tag="masked")
                nc.vector.tensor_mul(masked, s_choice, emask)

                # top-8 experts
                v8 = epi_pool.tile([P, 8], f32, tag="v8")
                nc.vector.max(out=v8, in_=masked)
                idx_u = epi_pool.tile([P, 8], u32, tag="idx_u")
                nc.vector.max_index(idx_u, v8, masked)

                # gather raw sigmoid scores at the top-8 positions by matching
                # each top value against the masked tensor (ties have ~0 prob)
                raw8 = epi_pool.tile([P, 8], f32, tag="raw8")
                for kk in range(TOP_K):
                    sc256 = epi_pool.tile([P, E], f32, tag="emask")
                    nc.vector.scalar_tensor_tensor(
                        out=sc256,
                        in0=masked,
                        scalar=v8[:, kk : kk + 1],
                        in1=scores,
                        op0=Alu.is_equal,
                        op1=Alu.mult,
                        accum_out=raw8[:, kk : kk + 1],
                    )

                # normalize * 2.5
                rsum = epi_pool.tile([P, 1], f32, tag="rsum")
                nc.vector.reduce_sum(rsum, raw8, axis=mybir.AxisListType.X)
                nc.vector.tensor_scalar(
                    rsum, rsum, 1.0 / ROUTED_SCALING, None, op0=Alu.mult
                )
                rcp = epi_pool.tile([P, 1], f32, tag="rcp")
                nc.vector.reciprocal(rcp, rsum)
                stage_w = epi_pool.tile([P, TOP_K], f32, tag="stage_w")
                nc.scalar.mul(stage_w, raw8, rcp)
                stage_i = epi_pool.tile([P, TOP_K], u32, tag="stage_i")
                nc.vector.tensor_copy(stage_i, idx_u)

                # ---- per-tile output DMA ----------------------------------
                nc.sync.dma_start(
                    out=out_w.ap().rearrange("(n p) k -> p n k", p=P)[:, ti, :],
                    in_=stage_w,
                )
                nc.sync.dma_start(
                    out=out_i.ap()
                    .rearrange("(n p) k -> p n k", p=P)
                    .bitcast(u32)[:, ti, :],
                    in_=stage_i,
                )

    nc.finalize()
    return nc


_NC_CACHE = {}


def _get_program():
    if "prog" not in _NC_CACHE:
        _NC_CACHE["prog"] = _build_program()
    return _NC_CACHE["prog"]


def kernel(hidden_states, weight, e_score_correction_bias, _trace=False):
    _ensure_path()
    from concourse.bass_utils import run_bass_kernel_spmd

    hidden_states = np.ascontiguousarray(hidden_states, dtype=np.float32)
    weight = np.ascontiguousarray(weight, dtype=np.float32)
    e_score_correction_bias = np.ascontiguousarray(
        e_score_correction_bias, dtype=np.float32
    )

    nc = _get_program()
    in_maps = [
        {
            "hidden_states": hidden_states[i * T_LOC : (i + 1) * T_LOC],
            "weight": weight,
            "e_score_correction_bias": e_score_correction_bias,
        }
        for i in range(N_CORES)
    ]
    res = run_bass_kernel_spmd(
        nc, in_maps, core_ids=list(range(N_CORES)), trace=_trace
    )
    routing_weights = np.concatenate(
        [res.results[i]["routing_weights"] for i in range(N_CORES)], axis=0
    )
    selected_experts = np.concatenate(
        [res.results[i]["selected_experts"] for i in range(N_CORES)], axis=0
    )
    if _trace:
        return (routing_weights, selected_experts), res
    return routing_weights, selected_experts
